# revision 6
# baseline (speedup 1.0000x reference)
"""EmbeddingBag(mean) over ragged char bags on 8 Trainium2 NeuronCores.

Problem: chars [1024, 256, 16] int32 (vocab 256), lengths [1024, 256] int32
in [1, 16], emb_table [256, 50] f32. Output [1024, 256, 50] f32 =
mean(emb_table[chars[b, s, :lengths[b, s]]]) per bag.

Strategy (data-parallel over batch, 128 batch rows -> 32768 bags per core):
  For each tile of 128 bags (bag per partition):
    1. DVE: 16 fused compare-accumulate (scalar_tensor_tensor) ops build
       counts[bag, class] (bf16, exact small ints), with padded slots
       redirected to an out-of-range sentinel class.
    2. PE: transpose counts -> [class, bag] (two 128x128 chunks).
    3. PE: out[bag, e] = sum_c counts[c, bag] * emb[c, e] as 4 accumulated
       matmuls (bf16 hi/lo split of the f32 table for f32-level accuracy).
    4. ScalarE: evict PSUM -> SBUF scaled by 1/length (per-partition scale).
"""

import os
import sys

sys.path.insert(0, "/opt/trn_rl_repo")
sys.path.insert(0, os.path.dirname(os.path.abspath(__file__)))

import numpy as np

import concourse.bacc as bacc
import concourse.bass as bass
from concourse import mybir
from concourse.bass_utils import run_bass_kernel_spmd
from concourse.masks import make_identity
from concourse.vector_clock import ScopedClock, VectorClock
import concourse.tile as tile

B, S, W = 1024, 256, 16
NB_CLASSES = 256
EMB = 50
N_CORES = 8

ROWS_PER_CORE = B // N_CORES          # 128 batch rows
TOK = ROWS_PER_CORE * S               # 32768 bags per core
SUBT = 4                              # 128-bag subtiles per macro tile
MACRO = TOK // (128 * SUBT)           # 64 macro tiles
SENTINEL = 300.0                      # masked chars compare against this

f32 = mybir.dt.float32
bf16 = mybir.dt.bfloat16
i32 = mybir.dt.int32
AF = mybir.ActivationFunctionType
ALU = mybir.AluOpType


class _TileContextFixed(tile.TileContext):
    """Work around walrus CoreV3 limit of one sync wait per CTRL
    instruction: split the kernel-tail drain's waits across SP nops."""

    def _drain_and_barrier(self, tick_clock, wait_clock):
        gc = tick_clock.global_clock
        n = len(gc)
        for i in [i for i in range(n) if gc[i] > 0]:
            vec = [0] * n
            vec[i] = gc[i]
            nop = self.nc.sync.nop(nofuse=True, hint="drain_split")
            wait_clock.add_sem_waits(nop.ins, ScopedClock({None: VectorClock(vec)}))
        self.nc.sync.drain()
        self.nc.all_engine_barrier()
        assert self.sems is not None
        popped = self.nc._tile_sem_poison_stack.pop()
        assert popped is self._sem_poison
        self.nc.clear_and_free_semaphores(list(self.sems.allocated().values()))
        self.nc.all_engine_barrier()


def build_program() -> bass.Bass:
    # Bacc (not plain Bass): its compile() pass `generate_event_semaphores`
    # splits multi-sem waits — this toolchain's walrus allows only one sync
    # wait per instruction.
    nc = bacc.Bacc()
    chars_d = nc.declare_dram_parameter("chars", [TOK, W], i32, isOutput=False)
    len_d = nc.declare_dram_parameter("lengths", [TOK], i32, isOutput=False)
    emb_d = nc.declare_dram_parameter("emb", [NB_CLASSES, EMB], f32, isOutput=False)
    out_d = nc.declare_dram_parameter("out", [TOK, EMB], f32, isOutput=True)

    chars_v = chars_d.rearrange("(mm s p) w -> mm p s w", s=SUBT, p=128)
    len_v = len_d.rearrange("(k p) -> p k", p=128)
    out_v = out_d.rearrange("(k p) e -> k p e", p=128)

    with tile.TileContext(nc) as tc:
        with (
            tc.tile_pool(name="singles", bufs=1) as singles,
            tc.tile_pool(name="chars", bufs=3) as chars_pool,
            tc.tile_pool(name="mask", bufs=3) as mask_pool,
            tc.tile_pool(name="acc", bufs=4) as acc_pool,
            tc.tile_pool(name="cnt", bufs=4) as cnt_pool,
            tc.tile_pool(name="osb", bufs=4) as osb_pool,
            tc.tile_pool(name="pt", bufs=2, space="PSUM") as pt_pool,
            tc.tile_pool(name="po", bufs=2, space="PSUM") as po_pool,
        ):
            # ---- one-time constants ----
            ident = singles.tile([128, 128], bf16)
            make_identity(nc, ident)

            iota_c_i = singles.tile([128, NB_CLASSES], i32)
            nc.gpsimd.iota(iota_c_i, pattern=[[1, NB_CLASSES]], channel_multiplier=0)
            iota_c = singles.tile([128, NB_CLASSES], bf16)
            nc.vector.tensor_copy(iota_c, iota_c_i)

            iota_w_i = singles.tile([128, SUBT, W], i32)
            nc.gpsimd.iota(iota_w_i, pattern=[[0, SUBT], [1, W]], channel_multiplier=0)
            iota_w = singles.tile([128, SUBT, W], f32)
            nc.vector.tensor_copy(iota_w, iota_w_i)

            # embedding chunks, bf16 hi + lo residual
            emb_f = singles.tile([128, 2, EMB], f32)
            nc.sync.dma_start(out=emb_f[:, 0, :], in_=emb_d[0:128, :])
            nc.sync.dma_start(out=emb_f[:, 1, :], in_=emb_d[128:256, :])
            emb_hi = singles.tile([128, 2, EMB], bf16)
            nc.vector.tensor_copy(emb_hi, emb_f)
            emb_hi_f = singles.tile([128, 2, EMB], f32)
            nc.vector.tensor_copy(emb_hi_f, emb_hi)
            emb_lo_f = singles.tile([128, 2, EMB], f32)
            nc.vector.tensor_tensor(
                out=emb_lo_f, in0=emb_f, in1=emb_hi_f, op=ALU.subtract
            )
            emb_lo = singles.tile([128, 2, EMB], bf16)
            nc.vector.tensor_copy(emb_lo, emb_lo_f)

            # lengths -> f32 and reciprocal, for all 256 column-tiles
            len_i = singles.tile([128, TOK // 128], i32)
            nc.sync.dma_start(out=len_i, in_=len_v)
            len_f = singles.tile([128, TOK // 128], f32)
            nc.vector.tensor_copy(len_f, len_i)
            inv_l = singles.tile([128, TOK // 128], f32)
            nc.vector.reciprocal(inv_l, len_f)

            # ---- main loop: 64 macro tiles x 4 subtiles of 128 bags ----
            for mm in range(MACRO):
                chars_i = chars_pool.tile([128, SUBT, W], i32)
                nc.sync.dma_start(out=chars_i, in_=chars_v[mm])

                cf = mask_pool.tile([128, SUBT, W], f32, tag="cf")
                nc.vector.tensor_copy(cf, chars_i)
                # m = (iota_w < len) per bag; len broadcast over W via step-0 AP
                lrep = bass.AP(
                    tensor=len_f.tensor,
                    offset=len_f.offset + mm * SUBT,
                    ap=[len_f.ap[0], [1, SUBT], [0, W]],
                )
                m = mask_pool.tile([128, SUBT, W], f32, tag="m")
                nc.vector.tensor_tensor(out=m, in0=iota_w, in1=lrep, op=ALU.is_lt)
                # cmask = m * (chars - SENTINEL) + SENTINEL
                t1 = mask_pool.tile([128, SUBT, W], f32, tag="t1")
                nc.vector.tensor_scalar_add(t1, cf, -SENTINEL)
                t2 = mask_pool.tile([128, SUBT, W], f32, tag="t2")
                nc.vector.tensor_tensor(out=t2, in0=t1, in1=m, op=ALU.mult)
                cm = mask_pool.tile([128, SUBT, W], f32, tag="cm")
                nc.vector.tensor_scalar_add(cm, t2, SENTINEL)

                for s in range(SUBT):
                    k = mm * SUBT + s  # 128-bag tile index
                    acc = acc_pool.tile([128, NB_CLASSES], bf16)
                    nc.vector.tensor_scalar(
                        out=acc,
                        in0=iota_c,
                        scalar1=cm[:, s, 0:1],
                        scalar2=None,
                        op0=ALU.is_equal,
                    )
                    for w in range(1, W):
                        nc.vector.scalar_tensor_tensor(
                            out=acc,
                            in0=iota_c,
                            scalar=cm[:, s, w : w + 1],
                            in1=acc,
                            op0=ALU.is_equal,
                            op1=ALU.add,
                        )

                    pt0 = pt_pool.tile([128, 128], bf16, tag="pt0")
                    nc.tensor.transpose(pt0, acc[:, 0:128], ident)
                    pt1 = pt_pool.tile([128, 128], bf16, tag="pt1")
                    nc.tensor.transpose(pt1, acc[:, 128:256], ident)
                    cnt0 = cnt_pool.tile([128, 128], bf16, tag="cnt0")
                    nc.scalar.copy(cnt0, pt0)
                    cnt1 = cnt_pool.tile([128, 128], bf16, tag="cnt1")
                    nc.scalar.copy(cnt1, pt1)

                    po = po_pool.tile([128, EMB], f32)
                    nc.tensor.matmul(
                        po, lhsT=cnt0, rhs=emb_hi[:, 0, :], start=True, stop=False
                    )
                    nc.tensor.matmul(
                        po, lhsT=cnt1, rhs=emb_hi[:, 1, :], start=False, stop=False
                    )
                    nc.tensor.matmul(
                        po, lhsT=cnt0, rhs=emb_lo[:, 0, :], start=False, stop=False
                    )
                    nc.tensor.matmul(
                        po, lhsT=cnt1, rhs=emb_lo[:, 1, :], start=False, stop=True
                    )

                    osb = osb_pool.tile([128, EMB], f32)
                    nc.scalar.activation(
                        osb, po, AF.Copy, scale=inv_l[:, k : k + 1]
                    )
                    nc.sync.dma_start(out=out_v[k], in_=osb)

    nc.finalize()
    return nc


_PROGRAM = None


def _get_program() -> bass.Bass:
    global _PROGRAM
    if _PROGRAM is None:
        _PROGRAM = build_program()
    return _PROGRAM


def run_on_hw(chars, lengths, emb_table, trace=False, **kw):
    nc = _get_program()
    in_maps = []
    for i in range(N_CORES):
        sl = slice(i * ROWS_PER_CORE, (i + 1) * ROWS_PER_CORE)
        in_maps.append(
            {
                "chars": np.ascontiguousarray(chars[sl].reshape(TOK, W)),
                "lengths": np.ascontiguousarray(lengths[sl].reshape(TOK)),
                "emb": np.ascontiguousarray(emb_table),
            }
        )
    res = run_bass_kernel_spmd(nc, in_maps, list(range(N_CORES)), trace=trace, **kw)
    out = np.concatenate(
        [res.results[i]["out"].reshape(ROWS_PER_CORE, S, EMB) for i in range(N_CORES)],
        axis=0,
    )
    return out, res


def kernel(chars, lengths, emb_table):
    out, _ = run_on_hw(chars, lengths, emb_table)
    return out


# revision 26
# speedup vs baseline: 2.8440x; 2.8440x over previous
"""EmbeddingBag(mean) over ragged char bags on 8 Trainium2 NeuronCores.

Problem: chars [1024, 256, 16] int32 (vocab 256), lengths [1024, 256] int32
in [1, 16], emb_table [256, 50] f32. Output [1024, 256, 50] f32 =
mean(emb_table[chars[b, s, :lengths[b, s]]]) per bag.

Strategy (data-parallel over batch, 128 batch rows -> 32768 bags per core).
Per 128-bag subtile, the gather is turned into a matmul against per-bag
class-count columns, built without any slow per-slot compare chains:

  1. Mask pad slots to an out-of-range sentinel, transpose the [bag, slot]
     char matrix to [slot, bag] (one PE transpose per 512-bag macro tile).
  2. Broadcast each slot row to all 128 partitions with K=1 ones-matmuls
     into PSUM ([part, slot, bag] slabs), evict once to SBUF as bf16.
  3. One tensor_scalar is_equal per (class-chunk, slot-half) with the
     per-partition scalar = partition class index (iota) -> one-hot slabs
     at the DVE's fastest (4x) mode.
  4. Merge slot pairs (one tensor_tensor add per chunk, DVE+GPSIMD), then
     accumulate the remaining 8 slabs per chunk into counts[class, bag]
     via identity-stationary matmuls (PSUM accumulation does the w-sum).
  5. counts^T chunks (bf16, exact small ints) are the matmul stationary
     against the bf16 hi/lo-split embedding table -> out[bag, emb] in PSUM.
  6. ScalarE evicts PSUM scaled by 1/length (per-partition scale).
"""

import os
import sys

sys.path.insert(0, "/opt/trn_rl_repo")
sys.path.insert(0, os.path.dirname(os.path.abspath(__file__)))

import numpy as np

import concourse.bacc as bacc
import concourse.bass as bass
from concourse import mybir
from concourse.bass_utils import run_bass_kernel_spmd
from concourse.masks import make_identity
import concourse.tile as tile

B, S, W = 1024, 256, 16
NB_CLASSES = 256
EMB = 50
N_CORES = 8

ROWS_PER_CORE = B // N_CORES          # 128 batch rows
TOK = ROWS_PER_CORE * S               # 32768 bags per core
SUBT = 4                              # 128-bag subtiles per macro tile
MACRO = TOK // (128 * SUBT)           # 64 macro tiles
SENTINEL = 300.0                      # masked chars compare against this
HALF_W = W // 2

f32 = mybir.dt.float32
bf16 = mybir.dt.bfloat16
i32 = mybir.dt.int32
AF = mybir.ActivationFunctionType
ALU = mybir.AluOpType


def build_program(loop_n: int | None = None) -> bass.Bass:
    # Bacc (not plain Bass): its compile() pass `generate_event_semaphores`
    # splits multi-sem waits — this toolchain's walrus allows only one sync
    # wait per instruction.
    nc = bacc.Bacc()
    chars_d = nc.declare_dram_parameter("chars", [TOK, W], i32, isOutput=False)
    len_d = nc.declare_dram_parameter("lengths", [TOK], i32, isOutput=False)
    emb_d = nc.declare_dram_parameter("emb", [NB_CLASSES, EMB], f32, isOutput=False)
    out_d = nc.declare_dram_parameter("out", [TOK, EMB], f32, isOutput=True)

    chars_v = chars_d.rearrange("(mm s p) w -> mm p s w", s=SUBT, p=128)
    len_v = len_d.rearrange("(k p) -> p k", p=128)
    out_v = out_d.rearrange("(k p) e -> k p e", p=128)

    with tile.TileContext(nc) as tc:
        with (
            tc.tile_pool(name="singles", bufs=1) as singles,
            tc.tile_pool(name="chars", bufs=3) as chars_pool,
            tc.tile_pool(name="mask", bufs=2) as mask_pool,
            tc.tile_pool(name="bcs", bufs=3) as bcs_pool,
            tc.tile_pool(name="oh", bufs=2) as oh_pool,
            tc.tile_pool(name="cnt", bufs=3) as cnt_pool,
            tc.tile_pool(name="osb", bufs=4) as osb_pool,
            tc.tile_pool(name="bc_ps", bufs=2, space="PSUM") as bc_ps,
            tc.tile_pool(name="cnt_ps", bufs=1, space="PSUM") as cnt_ps,
            tc.tile_pool(name="out_ps", bufs=2, space="PSUM") as out_ps,
        ):
            # ---- one-time constants ----
            ident = singles.tile([128, 128], bf16)
            make_identity(nc, ident)

            ones_row = singles.tile([1, 128], bf16)
            nc.vector.memset(ones_row, 1.0)

            iota_cols_i = singles.tile([128, 2], i32)
            nc.gpsimd.iota(iota_cols_i, pattern=[[128, 2]], channel_multiplier=1)
            iota_cols = singles.tile([128, 2], f32)
            nc.vector.tensor_copy(iota_cols, iota_cols_i)  # col j: p + 128*j

            iota_w_i = singles.tile([128, SUBT, W], i32)
            nc.gpsimd.iota(iota_w_i, pattern=[[0, SUBT], [1, W]], channel_multiplier=0)
            iota_w = singles.tile([128, SUBT, W], f32)
            nc.vector.tensor_copy(iota_w, iota_w_i)

            # embedding chunks, bf16 hi + lo residual
            emb_f = singles.tile([128, 2, EMB], f32)
            nc.sync.dma_start(out=emb_f[:, 0, :], in_=emb_d[0:128, :])
            nc.sync.dma_start(out=emb_f[:, 1, :], in_=emb_d[128:256, :])
            emb_hi = singles.tile([128, 2, EMB], bf16)
            nc.vector.tensor_copy(emb_hi, emb_f)
            emb_hi_f = singles.tile([128, 2, EMB], f32)
            nc.vector.tensor_copy(emb_hi_f, emb_hi)
            emb_lo_f = singles.tile([128, 2, EMB], f32)
            nc.vector.tensor_tensor(
                out=emb_lo_f, in0=emb_f, in1=emb_hi_f, op=ALU.subtract
            )
            emb_lo = singles.tile([128, 2, EMB], bf16)
            nc.vector.tensor_copy(emb_lo, emb_lo_f)

            # lengths -> f32 and reciprocal, for all 256 column-tiles
            len_i = singles.tile([128, TOK // 128], i32)
            nc.sync.dma_start(out=len_i, in_=len_v)
            len_f = singles.tile([128, TOK // 128], f32)
            nc.vector.tensor_copy(len_f, len_i)
            inv_l = singles.tile([128, TOK // 128], f32)
            nc.vector.reciprocal(inv_l, len_f)

            # ---- main loop: 64 macro tiles x 4 subtiles of 128 bags ----
            import contextlib
            loop_cm = (
                tc.For_i(0, loop_n, 1) if loop_n is not None
                else contextlib.nullcontext()
            )
            with loop_cm:
                _main_loop(
                    nc, tc, chars_pool, mask_pool, bcs_pool, oh_pool, cnt_pool,
                    osb_pool, bc_ps, cnt_ps, out_ps, chars_v, out_v, len_f,
                    iota_w, iota_cols, ident, emb_hi, emb_lo, inv_l,
                )

    nc.finalize()
    return nc


def _main_loop(nc, tc, chars_pool, mask_pool, bcs_pool, oh_pool, cnt_pool,
               osb_pool, bc_ps, cnt_ps, out_ps, chars_v, out_v, len_f,
               iota_w, iota_cols, ident, emb_hi, emb_lo, inv_l):
            for mm in range(MACRO):
                chars_i = chars_pool.tile([128, SUBT, W], i32)
                nc.sync.dma_start(out=chars_i, in_=chars_v[mm])

                cf = mask_pool.tile([128, SUBT, W], f32, tag="cf")
                nc.vector.tensor_copy(cf, chars_i)
                # m = (iota_w < len) per bag; len broadcast over W via step-0 AP
                lrep = bass.AP(
                    tensor=len_f.tensor,
                    offset=len_f.offset + mm * SUBT,
                    ap=[len_f.ap[0], [1, SUBT], [0, W]],
                )
                m = mask_pool.tile([128, SUBT, W], f32, tag="m")
                nc.vector.tensor_tensor(out=m, in0=iota_w, in1=lrep, op=ALU.is_lt)
                # cm = m * (chars - SENTINEL) + SENTINEL  (f32, then bf16)
                t1 = mask_pool.tile([128, SUBT, W], f32, tag="t1")
                nc.vector.tensor_scalar_add(t1, cf, -SENTINEL)
                t2 = mask_pool.tile([128, SUBT, W], f32, tag="t2")
                nc.vector.tensor_tensor(out=t2, in0=t1, in1=m, op=ALU.mult)
                cm = mask_pool.tile([128, SUBT, W], f32, tag="cm")
                nc.vector.tensor_scalar_add(cm, t2, SENTINEL)
                cmb = mask_pool.tile([128, SUBT, W], bf16, tag="cmb")
                nc.vector.tensor_copy(cmb, cm)

                for s in range(SUBT):
                    k = mm * SUBT + s  # 128-bag tile index

                    # chars broadcast: per slot, transpose a free-step-0
                    # column view of cmb -> [part, bag] slab (bf16 in PSUM)
                    bc_p = bc_ps.tile([128, W, 128], bf16, tag="bc")
                    for sl in range(W):
                        col_rep = bass.AP(
                            tensor=cmb.tensor,
                            offset=cmb.offset + s * W + sl,
                            ap=[cmb.ap[0], [0, 128]],
                        )
                        nc.tensor.transpose(bc_p[:, sl, :], col_rep, ident)
                    # evict broadcast slabs, split across ACT and DVE
                    bcsb = bcs_pool.tile([128, W, 128], bf16, tag="bcsb")
                    nc.scalar.copy(bcsb, bc_p)

                    # one-hot slabs per class chunk
                    cnts = []
                    for c in range(2):
                        oh0 = oh_pool.tile([128, W, 128], bf16,
                                           name=f"oh{c}", tag=f"oh{c}")
                        nc.vector.tensor_scalar(
                            out=oh0, in0=bcsb, scalar1=iota_cols[:, c : c + 1],
                            scalar2=None, op0=ALU.is_equal,
                        )
                        eng = nc.vector if c == 0 else nc.gpsimd
                        eng.tensor_tensor(
                            out=oh0[:, 0:HALF_W, :],
                            in0=oh0[:, 0:HALF_W, :],
                            in1=oh0[:, HALF_W:W, :],
                            op=ALU.add,
                        )
                        n_slabs = HALF_W
                        cnt_p = cnt_ps.tile([128, 128], f32, name=f"cntp{c}",
                                            tag=f"cntp{c}")
                        for sl in range(n_slabs):
                            nc.tensor.matmul(
                                cnt_p,
                                lhsT=ident,
                                rhs=oh0[:, sl, :],
                                start=(sl == 0),
                                stop=(sl == n_slabs - 1),
                            )
                        cnt = cnt_pool.tile([128, 128], bf16, name=f"cnt{c}",
                                            tag=f"cnt{c}")
                        nc.scalar.copy(cnt, cnt_p)
                        cnts.append(cnt)

                    po = out_ps.tile([128, EMB], f32)
                    n_mm = 0
                    for c in range(2):
                        for emb_t in (emb_hi, emb_lo):
                            nc.tensor.matmul(
                                po,
                                lhsT=cnts[c],
                                rhs=emb_t[:, c, :],
                                start=(n_mm == 0),
                                stop=(n_mm == 3),
                            )
                            n_mm += 1

                    osb = osb_pool.tile([128, EMB], f32)
                    nc.scalar.activation(
                        osb, po, AF.Copy, scale=inv_l[:, k : k + 1]
                    )
                    nc.sync.dma_start(out=out_v[k], in_=osb)


_PROGRAM = None


def _get_program() -> bass.Bass:
    global _PROGRAM
    if _PROGRAM is None:
        _PROGRAM = build_program()
    return _PROGRAM


def run_on_hw(chars, lengths, emb_table, trace=False, **kw):
    nc = _get_program()
    in_maps = []
    for i in range(N_CORES):
        sl = slice(i * ROWS_PER_CORE, (i + 1) * ROWS_PER_CORE)
        in_maps.append(
            {
                "chars": np.ascontiguousarray(chars[sl].reshape(TOK, W)),
                "lengths": np.ascontiguousarray(lengths[sl].reshape(TOK)),
                "emb": np.ascontiguousarray(emb_table),
            }
        )
    res = run_bass_kernel_spmd(nc, in_maps, list(range(N_CORES)), trace=trace, **kw)
    out = np.concatenate(
        [res.results[i]["out"].reshape(ROWS_PER_CORE, S, EMB) for i in range(N_CORES)],
        axis=0,
    )
    return out, res


def kernel(chars, lengths, emb_table):
    out, _ = run_on_hw(chars, lengths, emb_table)
    return out


# revision 29
# speedup vs baseline: 2.9839x; 1.0492x over previous
"""EmbeddingBag(mean) over ragged char bags on 8 Trainium2 NeuronCores.

Problem: chars [1024, 256, 16] int32 (vocab 256), lengths [1024, 256] int32
in [1, 16], emb_table [256, 50] f32. Output [1024, 256, 50] f32 =
mean(emb_table[chars[b, s, :lengths[b, s]]]) per bag.

Strategy (data-parallel over batch, 128 batch rows -> 32768 bags per core).
Per 128-bag subtile, the gather is turned into a matmul against per-bag
class-count columns, built without any slow per-slot compare chains:

  1. Mask pad slots to an out-of-range sentinel, transpose the [bag, slot]
     char matrix to [slot, bag] (one PE transpose per 512-bag macro tile).
  2. Broadcast each slot row to all 128 partitions with K=1 ones-matmuls
     into PSUM ([part, slot, bag] slabs), evict once to SBUF as bf16.
  3. One tensor_scalar is_equal per (class-chunk, slot-half) with the
     per-partition scalar = partition class index (iota) -> one-hot slabs
     at the DVE's fastest (4x) mode.
  4. Merge slot pairs (one tensor_tensor add per chunk, DVE+GPSIMD), then
     accumulate the remaining 8 slabs per chunk into counts[class, bag]
     via identity-stationary matmuls (PSUM accumulation does the w-sum).
  5. counts^T chunks (bf16, exact small ints) are the matmul stationary
     against the bf16 hi/lo-split embedding table -> out[bag, emb] in PSUM.
  6. ScalarE evicts PSUM scaled by 1/length (per-partition scale).
"""

import os
import sys

sys.path.insert(0, "/opt/trn_rl_repo")
sys.path.insert(0, os.path.dirname(os.path.abspath(__file__)))

import numpy as np

import concourse.bacc as bacc
import concourse.bass as bass
from concourse import mybir
from concourse.bass_utils import run_bass_kernel_spmd
from concourse.masks import make_identity
import concourse.tile as tile

B, S, W = 1024, 256, 16
NB_CLASSES = 256
EMB = 50
N_CORES = 8

ROWS_PER_CORE = B // N_CORES          # 128 batch rows
TOK = ROWS_PER_CORE * S               # 32768 bags per core
SUBT = 4                              # 128-bag subtiles per macro tile
MACRO = TOK // (128 * SUBT)           # 64 macro tiles
SENTINEL = 300.0                      # masked chars compare against this
HALF_W = W // 2

f32 = mybir.dt.float32
bf16 = mybir.dt.bfloat16
i32 = mybir.dt.int32
AF = mybir.ActivationFunctionType
ALU = mybir.AluOpType


def build_program(loop_n: int | None = None) -> bass.Bass:
    # Bacc (not plain Bass): its compile() pass `generate_event_semaphores`
    # splits multi-sem waits — this toolchain's walrus allows only one sync
    # wait per instruction.
    nc = bacc.Bacc()
    chars_d = nc.declare_dram_parameter("chars", [TOK, W], i32, isOutput=False)
    len_d = nc.declare_dram_parameter("lengths", [TOK], i32, isOutput=False)
    emb_d = nc.declare_dram_parameter("emb", [NB_CLASSES, EMB], f32, isOutput=False)
    out_d = nc.declare_dram_parameter("out", [TOK, EMB], f32, isOutput=True)

    chars_v = chars_d.rearrange("(mm s p) w -> mm p s w", s=SUBT, p=128)
    len_v = len_d.rearrange("(k p) -> p k", p=128)
    out_v = out_d.rearrange("(k p) e -> k p e", p=128)

    with tile.TileContext(nc) as tc:
        with (
            tc.tile_pool(name="singles", bufs=1) as singles,
            tc.tile_pool(name="chars", bufs=3) as chars_pool,
            tc.tile_pool(name="mask", bufs=2) as mask_pool,
            tc.tile_pool(name="bcs", bufs=3) as bcs_pool,
            tc.tile_pool(name="oh", bufs=2) as oh_pool,
            tc.tile_pool(name="cnt", bufs=3) as cnt_pool,
            tc.tile_pool(name="osb", bufs=4) as osb_pool,
            tc.tile_pool(name="bc_ps", bufs=2, space="PSUM") as bc_ps,
            tc.tile_pool(name="cnt_ps", bufs=1, space="PSUM") as cnt_ps,
            tc.tile_pool(name="out_ps", bufs=2, space="PSUM") as out_ps,
        ):
            # ---- one-time constants ----
            ident = singles.tile([128, 128], bf16)
            make_identity(nc, ident)

            ones_row = singles.tile([1, 128], bf16)
            nc.vector.memset(ones_row, 1.0)

            iota_cols_i = singles.tile([128, 2], i32)
            nc.gpsimd.iota(iota_cols_i, pattern=[[128, 2]], channel_multiplier=1)
            iota_cols = singles.tile([128, 2], f32)
            nc.vector.tensor_copy(iota_cols, iota_cols_i)  # col j: p + 128*j

            iota_w_i = singles.tile([128, SUBT, W], i32)
            nc.gpsimd.iota(iota_w_i, pattern=[[0, SUBT], [1, W]], channel_multiplier=0)
            iota_w = singles.tile([128, SUBT, W], f32)
            nc.vector.tensor_copy(iota_w, iota_w_i)

            # embedding chunks, bf16 hi + lo residual
            emb_f = singles.tile([128, 2, EMB], f32)
            nc.sync.dma_start(out=emb_f[:, 0, :], in_=emb_d[0:128, :])
            nc.sync.dma_start(out=emb_f[:, 1, :], in_=emb_d[128:256, :])
            emb_hi = singles.tile([128, 2, EMB], bf16)
            nc.vector.tensor_copy(emb_hi, emb_f)
            emb_hi_f = singles.tile([128, 2, EMB], f32)
            nc.vector.tensor_copy(emb_hi_f, emb_hi)
            emb_lo_f = singles.tile([128, 2, EMB], f32)
            nc.vector.tensor_tensor(
                out=emb_lo_f, in0=emb_f, in1=emb_hi_f, op=ALU.subtract
            )
            emb_lo = singles.tile([128, 2, EMB], bf16)
            nc.vector.tensor_copy(emb_lo, emb_lo_f)

            # lengths -> f32 and reciprocal, for all 256 column-tiles
            len_i = singles.tile([128, TOK // 128], i32)
            nc.sync.dma_start(out=len_i, in_=len_v)
            len_f = singles.tile([128, TOK // 128], f32)
            nc.vector.tensor_copy(len_f, len_i)
            inv_l = singles.tile([128, TOK // 128], f32)
            nc.vector.reciprocal(inv_l, len_f)

            # ---- main loop: 64 macro tiles x 4 subtiles of 128 bags ----
            import contextlib
            loop_cm = (
                tc.For_i(0, loop_n, 1) if loop_n is not None
                else contextlib.nullcontext()
            )
            with loop_cm:
                _main_loop(
                    nc, tc, chars_pool, mask_pool, bcs_pool, oh_pool, cnt_pool,
                    osb_pool, bc_ps, cnt_ps, out_ps, chars_v, out_v, len_f,
                    iota_w, iota_cols, ident, emb_hi, emb_lo, inv_l,
                )

    nc.finalize()
    return nc


def _main_loop(nc, tc, chars_pool, mask_pool, bcs_pool, oh_pool, cnt_pool,
               osb_pool, bc_ps, cnt_ps, out_ps, chars_v, out_v, len_f,
               iota_w, iota_cols, ident, emb_hi, emb_lo, inv_l):
            for mm in range(MACRO):
                chars_i = chars_pool.tile([128, SUBT, W], i32)
                nc.sync.dma_start(out=chars_i, in_=chars_v[mm])

                cf = mask_pool.tile([128, SUBT, W], f32, tag="cf")
                nc.vector.tensor_copy(cf, chars_i)
                # m = (iota_w < len) per bag; len broadcast over W via step-0 AP
                lrep = bass.AP(
                    tensor=len_f.tensor,
                    offset=len_f.offset + mm * SUBT,
                    ap=[len_f.ap[0], [1, SUBT], [0, W]],
                )
                m = mask_pool.tile([128, SUBT, W], f32, tag="m")
                nc.vector.tensor_tensor(out=m, in0=iota_w, in1=lrep, op=ALU.is_lt)
                # cm = m * (chars - SENTINEL) + SENTINEL  (f32, then bf16)
                t1 = mask_pool.tile([128, SUBT, W], f32, tag="t1")
                nc.vector.tensor_scalar_add(t1, cf, -SENTINEL)
                t2 = mask_pool.tile([128, SUBT, W], f32, tag="t2")
                nc.vector.tensor_tensor(out=t2, in0=t1, in1=m, op=ALU.mult)
                cm = mask_pool.tile([128, SUBT, W], f32, tag="cm")
                nc.vector.tensor_scalar_add(cm, t2, SENTINEL)
                cmb = mask_pool.tile([128, SUBT, W], bf16, tag="cmb")
                nc.vector.tensor_copy(cmb, cm)

                for s in range(SUBT):
                    k = mm * SUBT + s  # 128-bag tile index

                    # chars broadcast: per slot, transpose a free-step-0
                    # column view of cmb -> [part, bag] slab (bf16 in PSUM)
                    bc_p = bc_ps.tile([128, W, 128], bf16, tag="bc")
                    for sl in range(W):
                        col_rep = bass.AP(
                            tensor=cmb.tensor,
                            offset=cmb.offset + s * W + sl,
                            ap=[cmb.ap[0], [0, 128]],
                        )
                        nc.tensor.transpose(bc_p[:, sl, :], col_rep, ident)
                    # evict broadcast slabs, split across ACT and DVE
                    bcsb = bcs_pool.tile([128, W, 128], bf16, tag="bcsb")
                    nc.scalar.copy(bcsb[:, 0:14, :], bc_p[:, 0:14, :])
                    nc.vector.tensor_copy(bcsb[:, 14:16, :], bc_p[:, 14:16, :])

                    # one-hot slabs per class chunk
                    cnts = []
                    for c in range(2):
                        oh0 = oh_pool.tile([128, W, 128], bf16,
                                           name=f"oh{c}", tag=f"oh{c}")
                        nc.vector.tensor_scalar(
                            out=oh0, in0=bcsb, scalar1=iota_cols[:, c : c + 1],
                            scalar2=None, op0=ALU.is_equal,
                        )
                        eng = nc.vector if c == 0 else nc.gpsimd
                        eng.tensor_tensor(
                            out=oh0[:, 0:HALF_W, :],
                            in0=oh0[:, 0:HALF_W, :],
                            in1=oh0[:, HALF_W:W, :],
                            op=ALU.add,
                        )
                        n_slabs = HALF_W
                        cnt_p = cnt_ps.tile([128, 128], f32, name=f"cntp{c}",
                                            tag=f"cntp{c}")
                        for sl in range(n_slabs):
                            nc.tensor.matmul(
                                cnt_p,
                                lhsT=ident,
                                rhs=oh0[:, sl, :],
                                start=(sl == 0),
                                stop=(sl == n_slabs - 1),
                            )
                        cnt = cnt_pool.tile([128, 128], bf16, name=f"cnt{c}",
                                            tag=f"cnt{c}")
                        nc.scalar.copy(cnt, cnt_p)
                        cnts.append(cnt)

                    po = out_ps.tile([128, EMB], f32)
                    n_mm = 0
                    for c in range(2):
                        for emb_t in (emb_hi, emb_lo):
                            nc.tensor.matmul(
                                po,
                                lhsT=cnts[c],
                                rhs=emb_t[:, c, :],
                                start=(n_mm == 0),
                                stop=(n_mm == 3),
                            )
                            n_mm += 1

                    osb = osb_pool.tile([128, EMB], f32)
                    nc.scalar.activation(
                        osb, po, AF.Copy, scale=inv_l[:, k : k + 1]
                    )
                    nc.sync.dma_start(out=out_v[k], in_=osb)


_PROGRAM = None


def _get_program() -> bass.Bass:
    global _PROGRAM
    if _PROGRAM is None:
        _PROGRAM = build_program()
    return _PROGRAM


def run_on_hw(chars, lengths, emb_table, trace=False, **kw):
    nc = _get_program()
    in_maps = []
    for i in range(N_CORES):
        sl = slice(i * ROWS_PER_CORE, (i + 1) * ROWS_PER_CORE)
        in_maps.append(
            {
                "chars": np.ascontiguousarray(chars[sl].reshape(TOK, W)),
                "lengths": np.ascontiguousarray(lengths[sl].reshape(TOK)),
                "emb": np.ascontiguousarray(emb_table),
            }
        )
    res = run_bass_kernel_spmd(nc, in_maps, list(range(N_CORES)), trace=trace, **kw)
    out = np.concatenate(
        [res.results[i]["out"].reshape(ROWS_PER_CORE, S, EMB) for i in range(N_CORES)],
        axis=0,
    )
    return out, res


def kernel(chars, lengths, emb_table):
    out, _ = run_on_hw(chars, lengths, emb_table)
    return out
